# revision 1
# baseline (speedup 1.0000x reference)
"""Distributed decoder-attention kernel for 8 TRN2 NeuronCores.

Sharding: data-parallel over batch (2 groups of 4 cores) x tensor-parallel
over heads (4 heads/core). Per-chunk ReduceScatter after partial o-proj.
Self-contained: hardcodes all shapes; only needs concourse on sys.path.
"""

import sys

import numpy as np

try:
    import concourse.bass as bass  # noqa: F401
except ImportError:
    sys.path.insert(0, "/opt/trn_rl_repo")
    import concourse.bass as bass  # noqa: F401

import ml_dtypes

import concourse.bacc as bacc
import concourse.mybir as mybir
import concourse.tile as tile
from concourse.bass import AP
from concourse.bass_utils import run_bass_kernel_spmd

BF16 = mybir.dt.bfloat16
F32 = mybir.dt.float32
nbf16 = ml_dtypes.bfloat16

B, T, D, HD = 2, 2048, 1024, 64
H = D // HD          # 16 heads total
NCORES, TPG = 8, 4   # 8 cores, 4-way tensor parallel within each batch group
HPC = H // TPG       # 4 heads per core
CH = 512             # q chunk width
NCH = T // CH        # 4 chunks
KT = T // 128        # 16 token tiles
DTL = D // 128       # 8 d-tiles
EPS = 1e-6
RG = [[0, 1, 2, 3], [4, 5, 6, 7]]

EXP = mybir.ActivationFunctionType.Exp
LN = mybir.ActivationFunctionType.Ln
SQUARE = mybir.ActivationFunctionType.Square
COPY = mybir.ActivationFunctionType.Copy
FP8E4 = mybir.dt.float8e4
ADD = mybir.AluOpType.add
MULT = mybir.AluOpType.mult

_GAT_PATCHED = False


def _patch_act_tables():
    """Make every ACT func this kernel uses resolve to one table set, so the
    table-load pass emits a single load instead of thrashing between sets."""
    global _GAT_PATCHED
    if _GAT_PATCHED:
        return
    _GAT_PATCHED = True
    orig = bacc.get_activation_tables

    def patched(arch):
        tabs = orig(arch)
        keep = "natural_log_exp_and_others"
        if keep in tabs:
            funcs = set(tabs[keep])
            for name, s in tabs.items():
                if name != keep:
                    s.difference_update(funcs)
        return tabs

    bacc.get_activation_tables = patched


def build_nc(reps=1, phase="rs8"):
    _patch_act_tables()
    nc = bacc.Bacc("TRN2", target_bir_lowering=False, debug=False,
                   num_devices=NCORES)

    xT_d = nc.dram_tensor("xT", [D, T], BF16, kind="ExternalInput")
    wqk_d = nc.dram_tensor("wqkT", [D, 512], BF16, kind="ExternalInput")
    wv_d = nc.dram_tensor("wvT", [D, 256], BF16, kind="ExternalInput")
    wo_d = nc.dram_tensor("woT", [256, D], BF16, kind="ExternalInput")
    t1_d = nc.dram_tensor("T1", [128, T], BF16, kind="ExternalInput")
    t2_d = nc.dram_tensor("T2", [128, T], BF16, kind="ExternalInput")
    xres_d = nc.dram_tensor("xres", [CH, D], F32, kind="ExternalInput")
    boff_d = nc.dram_tensor("boff", [1, 4], mybir.dt.uint32, kind="ExternalInput")
    out_d = nc.dram_tensor("out", [CH, D], F32, kind="ExternalOutput")

    # inline consts
    kq = np.arange(128)
    trim_np = np.where(kq[:, None] > kq[None, :], 0.0, 1.0).astype(np.float32)
    trim_np = np.concatenate([trim_np, trim_np], 1)  # [128, 256] both heads
    trim_d = nc.inline_tensor(trim_np.astype(ml_dtypes.float8_e4m3), "trimask")
    ones8_np = np.zeros((128, 4, 8), np.float32)
    for ft in range(4):
        for s in range(2):
            ones8_np[64 * s:64 * (s + 1), ft, 2 * ft + s] = 1.0
    ones8_d = nc.inline_tensor(ones8_np.astype(nbf16), "ones8")
    onesK_d = nc.inline_tensor(np.ones((128, 1), nbf16), "onesK")
    psig_np = np.zeros((128, 128), np.float32)
    psig_np[np.arange(128) ^ 32, np.arange(128)] = 1.0
    psig_d = nc.inline_tensor(psig_np.astype(nbf16), "psig")
    sel2_np = np.zeros((2, 128), np.float32)
    sel2_np[0, 0:64] = 1.0
    sel2_np[1, 64:128] = 1.0
    sel2_d = nc.inline_tensor(sel2_np.astype(nbf16), "sel2")

    with tile.TileContext(nc) as tc:
        with (
            tc.tile_pool(name="persist", bufs=1) as P,
            tc.tile_pool(name="dram", bufs=NCH, space="DRAM") as DR,
        ):
            xT = [P.tile([128, T], BF16, tag=f"xT{i}", name=f"xT{i}")
                  for i in range(DTL)]
            wqk = [P.tile([128, 512], BF16, tag=f"wqk{i}", name=f"wqk{i}")
                   for i in range(DTL)]
            wv = [P.tile([128, 256], BF16, tag=f"wv{i}", name=f"wv{i}")
                  for i in range(DTL)]
            wo = [P.tile([128, D], BF16, tag=f"wo{i}", name=f"wo{i}")
                  for i in range(2)]
            T1 = P.tile([128, T], BF16, tag="T1", name="T1")
            T2 = P.tile([128, T], BF16, tag="T2", name="T2")
            qT = [P.tile([128, T], BF16, tag=f"qT{i}", name=f"qT{i}")
                  for i in range(2)]
            kTt = [P.tile([128, T], BF16, tag=f"kT{i}", name=f"kT{i}")
                   for i in range(2)]
            vsb = [P.tile([128, 2 * 272], FP8E4, tag=f"v{m}", name=f"v{m}")
                   for m in range(KT // 2)]
            attn = [P.tile([128, T], BF16, tag=f"attn{i}", name=f"attn{i}")
                    for i in range(2)]
            Rt = P.tile([128, KT], F32, tag="Rt", name="Rt")
            r_sb = [P.tile([8, 512], BF16, tag=f"r{qc}", name=f"r{qc}")
                    for qc in range(NCH)]
            lsb = [P.tile([128, 512], F32, tag=f"lsb{qc}", name=f"lsb{qc}")
                   for qc in range(NCH)]
            trim_s = P.tile([128, 256], FP8E4, tag="trim", name="trim")
            ones8_s = P.tile([128, 4, 8], BF16, tag="ones8", name="ones8")
            onesK_s = P.tile([128, 1], BF16, tag="onesK", name="onesK")
            psig_s = P.tile([128, 128], BF16, tag="psig", name="psig")
            sel2_s = P.tile([2, 128], BF16, tag="sel2", name="sel2")
            cEPS = P.tile([128, 1], F32, tag="cEPS", name="cEPS")
            nc.vector.memset(cEPS[:], EPS)
            c8EPS = P.tile([128, 1], F32, tag="c8EPS", name="c8EPS")
            nc.vector.memset(c8EPS[:], 8.0 * EPS)
            cM2 = P.tile([128, 1], F32, tag="cM2", name="cM2")
            nc.vector.memset(cM2[:], -2.0)

            rs_in = [DR.tile([512, D], BF16, tag="rsin", name="rsin")
                     for _ in range(NCH)]
            rs_in8 = [DR.tile([1024, D], BF16, tag="rsin8", name="rsin8")
                      for _ in range(NCH)]
            rs_out = [DR.tile([128, D], BF16, tag="rsout", name="rsout")
                      for _ in range(NCH)]
            rscr = [DR.tile([512], F32, tag="rscr", name="rscr")
                    for _ in range(NCH)]
            rdr = [DR.tile([8 * 512], BF16, tag="rdr", name="rdr")
                   for _ in range(NCH)]
            ldr = [DR.tile([4 * 512], BF16, tag="ldr", name="ldr")
                   for _ in range(NCH)]

            # ---- loads
            nc.sync.dma_start(trim_s[:], trim_d[:, :])
            nc.sync.dma_start(ones8_s[:], ones8_d[:, :, :])
            nc.sync.dma_start(onesK_s[:], onesK_d[:, :])
            nc.sync.dma_start(psig_s[:], psig_d[:, :])
            nc.sync.dma_start(sel2_s[:], sel2_d[:, :])
            for i in range(DTL):
                nc.sync.dma_start(xT[i][:], xT_d[128 * i:128 * (i + 1), :])
            for i in range(DTL):
                nc.sync.dma_start(wqk[i][:], wqk_d[128 * i:128 * (i + 1), :])
                nc.sync.dma_start(wv[i][:], wv_d[128 * i:128 * (i + 1), :])
            for i in range(2):
                nc.sync.dma_start(wo[i][:], wo_d[128 * i:128 * (i + 1), :])
            nc.sync.dma_start(T1[:], t1_d[:, :])
            nc.sync.dma_start(T2[:], t2_d[:, :])
            for m in range(KT // 2):
                nc.gpsimd.memset(vsb[m][:], 1.0)
            boff_sb = P.tile([1, 4], mybir.dt.uint32, tag="boff", name="boff")
            nc.sync.dma_start(boff_sb[:], boff_d[:, :])
            if phase == "rs8":
                zt = P.tile([128, D], BF16, tag="zt", name="zt")
                nc.gpsimd.memset(zt[:], 0.0)
                for qc in range(NCH):
                    for j in range(8):
                        nc.sync.dma_start(rs_in8[qc][128 * j:128 * (j + 1), :],
                                          zt[:])
            boffs = []
            for ms in range(4):
                tmp = nc.alloc_registers(f"boff{ms}")
                nc.regs_load(tmp, boff_sb[0:1, ms:ms + 1])
                boffs.append(nc.snap(tmp, donate=True, min_val=0, max_val=896))

            def emit_body(rep):
              def attention_chunk(qc, pool_s, pool_o, mp, sbufs, obufs, pbufs):
                    cs = slice(qc * CH, (qc + 1) * CH)
                    n_kb = 4 * qc + 4
                    lsb_t = lsb[qc]
                    nc.gpsimd.memset(lsb_t[:], 1.0)
                    for hp in range(2):
                        psA = pool_o.tile([65, 512], F32, tag="oA", name="oA",
                                          bufs=1)
                        psB = pool_o.tile([65, 512], F32, tag="oB", name="oB",
                                          bufs=1)

                        def emit_pv(Pt2, colG, g2):
                            for s, pso in ((0, psA), (1, psB)):
                                h = 2 * hp + s
                                lhs = vsb[g2][:].rearrange(
                                    "p (j b) -> p j b",
                                    b=272)[:, :, 68 * h:68 * h + 65]
                                rhs = Pt2[:].rearrange(
                                    "p (j b) -> p j b",
                                    b=1024)[:, :, 512 * s + colG:512 * (s + 1)]
                                nc.tensor.matmul(
                                    pso[:, colG:512], lhs, rhs,
                                    start=(g2 == 0), stop=(g2 == n_kb // 2 - 1),
                                    perf_mode=mybir.MatmulPerfMode.DoubleRow,
                                    skip_group_check=True)

                        prev = None
                        Pt2 = None
                        for kb in range(n_kb):
                            diag = kb - 4 * qc
                            col0 = 128 * diag if diag >= 0 else 0
                            colG = col0 if kb % 2 == 0 else 128 * (diag - 1) \
                                if diag >= 1 else 0
                            ps_s = pool_s.tile([128, 1024], F32, tag="s",
                                               name="s", bufs=sbufs)
                            for s in range(2):
                                rl = 64 * s
                                nc.tensor.matmul(
                                    ps_s[:, 512 * s + col0:512 * s + 512],
                                    kTt[hp][rl:rl + 64, 128 * kb:128 * (kb + 1)],
                                    qT[hp][rl:rl + 64,
                                           qc * CH + col0:(qc + 1) * CH])
                            if kb % 2 == 0:
                                Pt2 = mp.tile([128, 2048], FP8E4, tag="p",
                                              name="p", bufs=pbufs)
                            half = (kb % 2) * 1024
                            pv = ps_s[:].rearrange("p (s q) -> p s q", q=512)
                            ov = Pt2[:, half:half + 1024].rearrange(
                                "p (s q) -> p s q", q=512)[:, :, col0:512]
                            nc.scalar.activation(ov, pv[:, :, col0:512], EXP,
                                                 bias=cM2[:, :])
                            if diag >= 0:
                                sub = Pt2[:, half:half + 1024].rearrange(
                                    "p (s q) -> p s q",
                                    q=512)[:, :, col0:col0 + 128]
                                nc.vector.tensor_mul(
                                    sub, sub,
                                    trim_s[:].rearrange("p (s q) -> p s q",
                                                        q=128))
                                if kb % 2 == 1 and col0 > colG:
                                    gap = Pt2[:, half:half + 1024].rearrange(
                                        "p (s q) -> p s q",
                                        q=512)[:, :, colG:col0]
                                    nc.vector.memset(gap, 0.0)
                            if kb % 2 == 1:
                                if prev is not None:
                                    emit_pv(*prev)
                                prev = (Pt2, colG, kb // 2)
                        emit_pv(*prev)
                        for s, pso in ((0, psA), (1, psB)):
                            h = 2 * hp + s
                            nc.vector.tensor_copy(
                                lsb_t[32 * h:32 * h + 1, :], pso[64:65, :])
                            nc.vector.tensor_copy(
                                attn[hp][64 * s:64 * (s + 1), cs], pso[0:64, :])

              with (
                tc.tile_pool(name=f"ps1_{rep}", bufs=2, space="PSUM") as PS1,
                tc.tile_pool(name=f"m1_{rep}", bufs=4) as M1,
              ):
                # ---- R = rsqrt(mean(x^2)+eps) per token, laid out [128, 16]
                def r2_all():
                    for qc in range(NCH):
                        cs = slice(qc * CH, (qc + 1) * CH)
                        ps_r2 = PS1.tile([1, 512], F32, tag="ss", name="ss",
                                         bufs=1)
                        for i in range(DTL):
                            x2 = M1.tile([128, 512], BF16, tag="x2", name="x2")
                            nc.scalar.activation(x2[:], xT[i][:, cs], SQUARE)
                            nc.tensor.matmul(ps_r2[:], onesK_s[:], x2[:],
                                             start=(i == 0), stop=(i == DTL - 1))
                        rln = M1.tile([1, 512], F32, tag="rln1", name="rln1")
                        nc.scalar.activation(rln[:], ps_r2[:], LN,
                                             bias=cEPS[0:1, :], scale=1.0 / D)
                        rr = M1.tile([1, 512], F32, tag="rr1", name="rr1")
                        nc.scalar.activation(rr[:], rln[:], EXP, scale=-0.5)
                        nc.gpsimd.dma_start(out=rscr[qc][:], in_=rr[0:1, :])
                        nc.gpsimd.dma_start(
                            out=Rt[:, 4 * qc:4 * (qc + 1)],
                            in_=rscr[qc][:].rearrange("(m p) -> p m", p=128))

                # ---- q/k projection + per-head rms factors + rope
                def qk_chunk(qc):
                    cs = slice(qc * CH, (qc + 1) * CH)
                    ps_ss = PS1.tile([8, 512], F32, tag="ss", name="ss", bufs=1)
                    raws = []
                    for ft in range(4):
                        ps_qk = PS1.tile([128, 512], F32, tag="qk", name="qk", bufs=3)
                        for i in range(DTL):
                            nc.tensor.matmul(
                                ps_qk[:], wqk[i][:, 128 * ft:128 * (ft + 1)],
                                xT[i][:, cs],
                                start=(i == 0), stop=(i == DTL - 1))
                        raw = M1.tile([128, 512], BF16, tag=f"raw{ft}",
                                      name=f"raw{ft}")
                        nc.vector.tensor_copy(raw[:], ps_qk[:])
                        sq = M1.tile([128, 512], BF16, tag="sq", name="sq")
                        nc.vector.tensor_mul(sq[:], raw[:], raw[:])
                        nc.tensor.matmul(ps_ss[:], ones8_s[:, ft, :], sq[:],
                                         start=(ft == 0), stop=(ft == 3))
                        raws.append(raw)
                    # r = rsqrt(ss/64+eps)/sqrt(8) (folds attention 1/sqrt(hd))
                    rln = M1.tile([8, 512], F32, tag="rln8", name="rln8")
                    nc.scalar.activation(rln[:], ps_ss[:], LN,
                                         bias=c8EPS[0:8, :], scale=8.0 / 64.0)
                    nc.scalar.activation(r_sb[qc][:], rln[:], EXP, scale=-0.5)
                    nc.sync.dma_start(
                        rdr[qc][:].rearrange("(h q) -> h q", q=512), r_sb[qc][:])
                    r2_sb = M1.tile([2, 4, 512], BF16, tag="r2sb", name="r2sb")
                    nc.sync.dma_start(
                        r2_sb[:],
                        AP(rdr[qc].tensor, rdr[qc].offset,
                           [[512, 2], [1024, 4], [1, 512]]))
                    for ft in range(4):
                        raw = raws[ft]
                        ps_sw = PS1.tile([128, 512], F32, tag="bc", name="bc", bufs=2)
                        nc.tensor.matmul(ps_sw[:], psig_s[:], raw[:])
                        t1t = M1.tile([128, 512], BF16, tag="t1t", name="t1t")
                        nc.vector.tensor_mul(t1t[:], raw[:], T1[:, cs])
                        t2t = M1.tile([128, 512], BF16, tag="t2t", name="t2t")
                        nc.vector.tensor_mul(t2t[:], ps_sw[:], T2[:, cs])
                        rope = M1.tile([128, 512], BF16, tag="rope", name="rope")
                        nc.gpsimd.tensor_add(rope[:], t1t[:], t2t[:])
                        # broadcast r rows (2ft, 2ft+1) to 64 partitions each
                        ps_rb = PS1.tile([128, 512], F32, tag="bc", name="bc", bufs=2)
                        nc.tensor.matmul(ps_rb[:], sel2_s[:], r2_sb[:, ft, :])
                        dst = (qT if ft < 2 else kTt)[ft % 2]
                        nc.vector.tensor_mul(dst[:, cs], rope[:], ps_rb[:])

                def v_group(ms):
                    for m in ms:
                        ps_v = PS1.tile([128, 256], F32, tag="v", name="v")
                        for i in range(DTL):
                            nc.tensor.matmul(
                                ps_v[:], xT[i][:, 128 * m:128 * (m + 1)],
                                wv[i][:],
                                start=(i == 0), stop=(i == DTL - 1))
                        vdst = vsb[m // 2][:, 272 * (m % 2):272 * (m % 2) + 272] \
                            .rearrange("p (h e) -> p h e", e=68)[:, :, 0:64]
                        nc.vector.tensor_scalar(
                            vdst, ps_v[:].rearrange("p (h e) -> p h e", e=64),
                            Rt[:, m:m + 1], None, MULT)

                r2_all()
                qk_chunk(0)
                v_group(range(0, 4))
                qk_chunk(1)
                v_group(range(4, 8))
                qk_chunk(2)
                v_group(range(8, 12))
                qk_chunk(3)
                v_group(range(12, 16))

              if phase == "prologue":
                  return
              # ---- attention + per-chunk partial o-proj + ReduceScatter
              with (
                tc.tile_pool(name=f"pss_{rep}", bufs=2, space="PSUM") as PSS,
                tc.tile_pool(name=f"pso_{rep}", bufs=2, space="PSUM") as PSO,
                tc.tile_pool(name=f"m2_{rep}", bufs=4) as M2,
              ):
                def tail_chunk(qc):
                    # linv = 1/l, broadcast, normalize, partial o-proj, RS
                    cs = slice(qc * CH, (qc + 1) * CH)
                    lsb_t = lsb[qc]
                    lnl = M2.tile([128, 512], F32, tag="lnl", name="lnl", bufs=1)
                    nc.scalar.activation(lnl[:], lsb_t[:], LN)
                    linv = M2.tile([128, 512], BF16, tag="linv", name="linv",
                                   bufs=1)
                    nc.scalar.activation(linv[:], lnl[:], EXP, scale=-1.0)
                    for h in range(4):
                        nc.sync.dma_start(
                            ldr[qc][512 * h:512 * (h + 1)],
                            linv[32 * h:32 * h + 1, :])
                    linv4 = M2.tile([2, 2, 512], BF16, tag="linv4", name="linv4")
                    nc.sync.dma_start(
                        linv4[:],
                        AP(ldr[qc].tensor, ldr[qc].offset,
                           [[512, 2], [1024, 2], [1, 512]]))
                    ans = []
                    for ft in range(2):
                        ps_lb = PSS.tile([128, 512], F32, tag="op", name="op", bufs=2)
                        nc.tensor.matmul(ps_lb[:], sel2_s[:], linv4[:, ft, :])
                        an = M2.tile([128, 512], BF16, tag=f"an{ft}",
                                     name=f"an{ft}")
                        nc.vector.tensor_mul(an[:], attn[ft][:, cs], ps_lb[:])
                        ans.append(an)
                    for ms in range(4):
                        for n in range(2):
                            ps_op = PSS.tile([128, 512], F32, tag="op",
                                             name="op", bufs=2)
                            for ft in range(2):
                                nc.tensor.matmul(
                                    ps_op[:],
                                    ans[ft][:, 128 * ms:128 * (ms + 1)],
                                    wo[ft][:, 512 * n:512 * (n + 1)],
                                    start=(ft == 0), stop=(ft == 1))
                            po = M2.tile([128, 512], BF16, tag="po", name="po")
                            nc.vector.tensor_copy(po[:], ps_op[:])
                            if phase == "rs8":
                                nc.sync.dma_start(
                                    rs_in8[qc][bass.ds(boffs[ms], 128),
                                               512 * n:512 * (n + 1)], po[:])
                            else:
                                nc.sync.dma_start(
                                    rs_in[qc][128 * ms:128 * (ms + 1),
                                              512 * n:512 * (n + 1)], po[:])
                    if phase == "nocc":
                        nc.gpsimd.dma_start(out=rs_out[qc][:, :],
                                            in_=rs_in[qc][0:128, :])
                    elif phase == "rs8":
                        nc.gpsimd.collective_compute(
                            "ReduceScatter", ADD,
                            replica_groups=[list(range(8))],
                            ins=[rs_in8[qc][:, :].opt()],
                            outs=[rs_out[qc][:, :].opt()])
                    else:
                        nc.gpsimd.collective_compute(
                            "ReduceScatter", ADD, replica_groups=RG,
                            ins=[rs_in[qc][:, :].opt()],
                            outs=[rs_out[qc][:, :].opt()])

                def post_rs(qc):
                    o_sb = M2.tile([128, D], BF16, tag="osb", name="osb")
                    nc.sync.dma_start(o_sb[:], rs_out[qc][:, :])
                    scr = M2.tile([128, D], BF16, tag="scr", name="scr")
                    ssum = M2.tile([128, 1], F32, tag="ssum", name="ssum")
                    nc.scalar.activation(scr[:], o_sb[:], SQUARE,
                                         accum_out=ssum[:])
                    lnm = M2.tile([128, 1], F32, tag="lnm", name="lnm")
                    nc.scalar.activation(lnm[:], ssum[:], LN,
                                         bias=1.0, scale=1.0 / D)
                    sc = M2.tile([128, 1], F32, tag="sc", name="sc")
                    nc.scalar.activation(sc[:], lnm[:], EXP, scale=-0.5)
                    xres_t = M2.tile([128, D], F32, tag="xrest", name="xrest")
                    nc.sync.dma_start(xres_t[:],
                                      xres_d[128 * qc:128 * (qc + 1), :])
                    res = M2.tile([128, D], F32, tag="res", name="res")
                    nc.vector.tensor_add(res[:], o_sb[:], xres_t[:])
                    nc.vector.tensor_scalar(res[:], res[:], sc[:], None, MULT)
                    nc.sync.dma_start(out_d[128 * qc:128 * (qc + 1), :], res[:])

                # attention runs ahead of its tail; tails/posts overlap
                # later chunks' exp streams
                attention_chunk(0, PSS, PSO, M2, 2, 2, 6)
                attention_chunk(1, PSS, PSO, M2, 2, 2, 6)
                tail_chunk(0)
                attention_chunk(2, PSS, PSO, M2, 2, 2, 6)
                tail_chunk(1)
                post_rs(0)
                tail_chunk(2)
                attention_chunk(3, PSS, PSO, M2, 2, 2, 6)
                post_rs(1)
                post_rs(2)
                tail_chunk(3)
                post_rs(3)

            for rep in range(reps):
                emit_body(rep)

    nc.compile()
    return nc


_NC_CACHE = {}


def _get_nc():
    if "nc" not in _NC_CACHE:
        _NC_CACHE["nc"] = build_nc()
    return _NC_CACHE["nc"]


def _make_in_maps(input_BTD, cos, sin, qkv_weight, temp_invm1, o_weight):
    inp = np.asarray(input_BTD, np.float32)
    cos = np.asarray(cos, np.float32)
    sin = np.asarray(sin, np.float32)
    qkvw = np.asarray(qkv_weight, np.float32)
    o_w = np.asarray(o_weight, np.float32)
    s = np.sqrt(np.asarray(temp_invm1, np.float64) + 1.0).astype(np.float32)
    s1, s2 = s[:32, None], s[32:, None]
    cosT, sinT = cos.T, sin.T  # [32, T]
    T1 = np.concatenate([s1 * cosT, s2 * cosT] * 2, 0).astype(nbf16)
    T2 = np.concatenate([-(s2 * sinT), s1 * sinT] * 2, 0).astype(nbf16)
    woT = o_w.T  # [D_in, D_out]
    xT = [np.ascontiguousarray(inp[b].T).astype(nbf16) for b in range(B)]
    maps = []
    for c in range(NCORES):
        b, g = divmod(c, TPG)
        rows = slice(256 * g, 256 * (g + 1))
        wqkT = np.ascontiguousarray(
            np.concatenate([qkvw[0][rows], qkvw[1][rows]], 0).T).astype(nbf16)
        wvT = np.ascontiguousarray(qkvw[2][rows].T).astype(nbf16)
        woTc = np.ascontiguousarray(woT[rows]).astype(nbf16)
        idx = (np.arange(NCH)[:, None] * CH + 128 * g
               + np.arange(128)[None, :]).reshape(-1)
        maps.append({
            "xT": xT[b], "wqkT": wqkT, "wvT": wvT, "woT": woTc,
            "T1": T1, "T2": T2,
            "xres": np.ascontiguousarray(inp[b][idx]),
            "boff": np.array([[512 * b, 512 * b + 128, 512 * b + 256,
                               512 * b + 384]], np.uint32),
        })
    return maps


LAST_RESULT = None


def kernel(input_BTD, cos, sin, qkv_weight, temp_invm1, o_weight, **kw):
    global LAST_RESULT
    nc = _get_nc()
    maps = _make_in_maps(input_BTD, cos, sin, qkv_weight, temp_invm1, o_weight)
    res = run_bass_kernel_spmd(nc, maps, core_ids=list(range(NCORES)), **kw)
    LAST_RESULT = res
    out = np.zeros((B, T, D), np.float32)
    for c in range(NCORES):
        b, g = divmod(c, TPG)
        r = np.asarray(res.results[c]["out"], np.float32)
        for qc in range(NCH):
            out[b, qc * CH + 128 * g: qc * CH + 128 * (g + 1), :] = \
                r[128 * qc:128 * (qc + 1), :]
    return out


if __name__ == "__main__":
    build_nc()
    print("build+compile OK")

